# revision 32
# baseline (speedup 1.0000x reference)
"""Trainium2 8-core attention kernel (nn_Attention_19954418057485).

Sharding: heads are split across the 8 cores (2 heads = 128 channels
each); every core processes both batch elements for its heads.  Two
AllToAlls over all 8 cores (one per batch element, each overlapped with
compute) swap the channel axis for the row axis, so each core finishes
the full output projection for 512 rows (256 from each batch) of the
flattened (B*N, C) output.

Per-core pipeline (matmuls on PE in bf16, exp on ACT, elementwise DVE):
  x^T (bf16)  --PE-->  q,k (rows,ch) + v^T        [QKV projection]
  q,k: LayerNorm (d=64) + RoPE (bf16 DVE ops), then PE transposes to
  q^T,k^T [ch, n]; v^T -> V [n, ch] with a ones column appended.
  per (batch, head): S^T = K Q^T, exp(S/8) on ACT (no max-subtraction
  needed: layernormed q,k bound |scores| <= 8), AV accumulates
  V_ext^T @ expS^T giving out^T rows 0..63 plus the softmax denominator
  in row 64 (from the ones column).  Normalization: denominator row ->
  PE outer-product broadcast -> 1/x via ACT exp(-ln(x)) (same ACT table
  set as Exp; DVE reciprocal is 3.4us/op) -> one DVE multiply.

Instruction emission interleaves batch-0 attention (ACT-bound) with the
batch-1 preamble (DVE-bound) and batch-1 attention with the first half
of the output projection, keeping every engine's in-order queue busy.
"""
import sys

if "/opt/trn_rl_repo" not in sys.path:
    sys.path.insert(0, "/opt/trn_rl_repo")

import numpy as np
import ml_dtypes

import concourse.bass as bass
import concourse.tile as tile
from concourse import mybir
from concourse.bass_utils import run_bass_kernel_spmd

N_CORES = 8
B, N, C, H = 2, 2048, 1024, 16
D = 64
HPC = H // N_CORES          # heads per core = 2
CPC = HPC * D               # channels per core = 128
NTOT = B * N                # 4096 flattened rows
RPC = NTOT // N_CORES       # output rows per core = 512
HALF = RPC // 2             # rows per core per batch = 256
EPS = 1e-6

BF16 = mybir.dt.bfloat16
F32 = mybir.dt.float32
AF = mybir.ActivationFunctionType
OP = mybir.AluOpType
AX = mybir.AxisListType


def _split_excess_waits(nc, max_waits=1):
    """walrus rejects instructions with more than a couple of sem-wait
    commands; split extras onto preceding same-engine NoOps."""
    for fn in nc.m.functions:
        for blk in fn.blocks:
            new_insts = []
            for ins in blk.instructions:
                si = ins.sync_info
                ow = list(si.on_wait) if si is not None and si.on_wait else []
                if len(ow) > max_waits:
                    head = ow[: len(ow) - max_waits]
                    rest = ow[len(ow) - max_waits:]
                    for i in range(0, len(head), max_waits):
                        new_insts.append(mybir.InstNoOp(
                            name=f"{ins.name}_ws{i}",
                            engine=ins.engine,
                            ins=[], outs=[],
                            sync_info=mybir.SyncInfo(
                                on_wait=head[i:i + max_waits], on_update=[]),
                        ))
                    ins.sync_info = mybir.SyncInfo(
                        on_wait=rest, on_update=list(si.on_update or []))
                new_insts.append(ins)
            blk.instructions = new_insts


def build():
    nc = bass.Bass("TRN2", target_bir_lowering=False, debug=False,
                   num_devices=N_CORES)
    xT_d = nc.dram_tensor("xT", (C, NTOT), BF16, kind="ExternalInput")
    wqkv_d = nc.dram_tensor("wqkvT", (C, 3 * CPC), BF16, kind="ExternalInput")
    wpT_d = nc.dram_tensor("wpT", (C, C), BF16, kind="ExternalInput")
    bias_d = nc.dram_tensor("biasb", (128, C), F32, kind="ExternalInput")
    cos_d = nc.dram_tensor("cosd", (N, D), BF16, kind="ExternalInput")
    sin_d = nc.dram_tensor("sind", (N, D), BF16, kind="ExternalInput")
    ident_d = nc.dram_tensor("identd", (128, 128), F32, kind="ExternalInput")
    out_d = nc.dram_tensor("out", (RPC, C), F32, kind="ExternalOutput")

    with tile.TileContext(nc) as tc:
        with tc.tile_pool(name="consts", bufs=1) as consts, \
             tc.tile_pool(name="xload", bufs=2) as xload, \
             tc.tile_pool(name="qkrp", bufs=2) as qkrp, \
             tc.tile_pool(name="freqs", bufs=2) as freqs, \
             tc.tile_pool(name="work", bufs=3) as work, \
             tc.tile_pool(name="small", bufs=2) as small, \
             tc.tile_pool(name="exps", bufs=6) as expp, \
             tc.tile_pool(name="norm", bufs=2) as normp, \
             tc.tile_pool(name="ps", bufs=2, space="PSUM") as ps, \
             tc.tile_pool(name="psS", bufs=2, space="PSUM") as psSp, \
             tc.tile_pool(name="psav", bufs=2, space="PSUM") as psav, \
             tc.tile_pool(name="dram", bufs=1, space="DRAM") as dram:

            # ---- constants -------------------------------------------------
            wqkv_sb = consts.tile([128, 8, 3 * CPC], BF16)
            nc.sync.dma_start(wqkv_sb[:],
                              wqkv_d.ap().rearrange("(co p) k -> p co k", p=128))
            wp_sb = consts.tile([128, 8, C], BF16)      # DMA deferred
            bias_sb = consts.tile([128, C], F32)
            nc.sync.dma_start(bias_sb[:], bias_d.ap())
            cos_r = cos_d.ap().rearrange("(c p) d -> p c d", p=128)
            sin_r = sin_d.ap().rearrange("(c p) d -> p c d", p=128)
            ident_f = consts.tile([128, 128], F32)
            nc.sync.dma_start(ident_f[:], ident_d.ap())
            identr = consts.tile([128, 128], BF16)
            nc.scalar.activation(identr[:], ident_f[:], AF.Copy)
            onesr = consts.tile([1, 64], BF16)
            nc.scalar.activation(onesr[:], ident_f[0:1, 0:64], AF.Identity,
                                 scale=0.0, bias=1.0)

            # ---- persistent tensors ---------------------------------------
            qkT = consts.tile([128, 2, NTOT], BF16)   # [ch, {q,k}, b*N+n]
            # V with ones column: [n%128, chunk, head, 64 d + 1 one + pad]
            vext = consts.tile([128, 32, HPC, 66], BF16)
            nc.scalar.activation(
                vext[:, :, :, 64:65],
                bias_sb[:, 0:64].rearrange("p (a b c) -> p a b c", a=32, b=2),
                AF.Identity, scale=0.0, bias=1.0)
            attn_outT = consts.tile([128, NTOT], BF16)

            xT_r = xT_d.ap().rearrange("(co p) n -> p co n", p=128)
            state = {}

            def pre_gen(tp):
                """QKV proj + LN + RoPE for rows [tp*1024, (tp+1)*1024)."""
                xt = xload.tile([128, 8, 1024], BF16, tag="xt",
                                name=f"xt{tp}")
                nc.sync.dma_start(xt[:], xT_r[:, :, 1024 * tp:1024 * (tp + 1)])
                qk_nd = work.tile([128, 8, 4, D], BF16, tag="qknd",
                                  name=f"qknd{tp}")
                for ns in range(8):
                    pj = ps.tile([128, 3 * CPC], F32, tag="ps",
                                 name=f"pj{tp}_{ns}")
                    for cc in range(8):
                        nc.tensor.matmul(pj[:],
                                         xt[:, cc, 128 * ns:128 * (ns + 1)],
                                         wqkv_sb[:, cc, :],
                                         start=(cc == 0), stop=(cc == 7))
                    if tp == 0:
                        nc.scalar.activation(
                            qk_nd[:, ns],
                            pj[:, 0:2 * CPC].rearrange("p (s d) -> p s d",
                                                       s=4), AF.Copy)
                        nc.scalar.activation(
                            vext[:, 8 * tp + ns, :, 0:64],
                            pj[:, 2 * CPC:3 * CPC].rearrange(
                                "p (h d) -> p h d", h=HPC), AF.Copy)
                    else:
                        nc.vector.tensor_copy(
                            qk_nd[:, ns],
                            pj[:, 0:2 * CPC].rearrange("p (s d) -> p s d",
                                                       s=4))
                        nc.vector.tensor_copy(
                            vext[:, 8 * tp + ns, :, 0:64],
                            pj[:, 2 * CPC:3 * CPC].rearrange(
                                "p (h d) -> p h d", h=HPC))
                    if ns == 3:
                        yield
                yield
                # LayerNorm stats over d=64 for each (row, slot)
                s1 = small.tile([128, 8, 4], F32, tag="s1", name=f"s1_{tp}")
                nc.vector.reduce_sum(s1[:], qk_nd[:], axis=AX.X)
                sq = work.tile([128, 8, 4, D], BF16, tag="tmp",
                               name=f"sq{tp}")
                nc.scalar.square(sq[:], qk_nd[:])
                s2 = small.tile([128, 8, 4], F32, tag="s2", name=f"s2_{tp}")
                nc.vector.reduce_sum(s2[:], sq[:], axis=AX.X)
                mu = small.tile([128, 8, 4], F32, tag="mu", name=f"mu{tp}")
                nc.vector.tensor_scalar_mul(mu[:], s1[:], 1.0 / D)
                var = small.tile([128, 8, 4], F32, tag="var", name=f"var{tp}")
                nc.vector.tensor_scalar_mul(var[:], s2[:], 1.0 / D)
                mm = small.tile([128, 8, 4], F32, tag="mm", name=f"mm{tp}")
                nc.vector.tensor_tensor(mm[:], mu[:], mu[:], OP.mult)
                nc.vector.tensor_tensor(var[:], var[:], mm[:], OP.subtract)
                nc.vector.tensor_scalar_add(var[:], var[:], EPS)
                # rsqrt(var+eps) = exp(-0.5*ln(var+eps)) on ACT: stays in
                # the ln/exp table set (no thrash against attention's Exp)
                lnv = small.tile([128, 8, 4], F32, tag="lnv", name=f"lnv{tp}")
                nc.scalar.activation(lnv[:], var[:], AF.Ln)
                a_ = small.tile([128, 8, 4], BF16, tag="a", name=f"a{tp}")
                nc.scalar.activation(a_[:], lnv[:], AF.Exp, scale=-0.5)
                nma = small.tile([128, 8, 4], BF16, tag="nma", name=f"nma{tp}")
                nc.vector.tensor_tensor(nma[:], mu[:], a_[:], OP.mult)
                yield
                # qn = q*a - mu*a
                nc.vector.tensor_tensor(
                    qk_nd[:], qk_nd[:],
                    a_[:, :, :, None].to_broadcast((128, 8, 4, D)), OP.mult)
                nc.vector.tensor_tensor(
                    qk_nd[:], qk_nd[:],
                    nma[:, :, :, None].to_broadcast((128, 8, 4, D)),
                    OP.subtract)
                yield
                # RoPE: out = qn*cos + rot_half(qn)*sin
                cs_lo = 8 * (tp % 2)
                cos_t = freqs.tile([128, 8, D], BF16, tag="cos",
                                   name=f"cos{tp}")
                nc.sync.dma_start(cos_t[:], cos_r[:, cs_lo:cs_lo + 8, :])
                sin_t = freqs.tile([128, 8, D], BF16, tag="sin",
                                   name=f"sin{tp}")
                nc.sync.dma_start(sin_t[:], sin_r[:, cs_lo:cs_lo + 8, :])
                cs = cos_t[:, :, None, :].to_broadcast((128, 8, 4, D))
                sn0 = sin_t[:, :, None, 0:32].to_broadcast((128, 8, 4, 32))
                sn1 = sin_t[:, :, None, 32:64].to_broadcast((128, 8, 4, 32))
                tmp = work.tile([128, 8, 4, D], BF16, tag="tmp",
                                name=f"tmp{tp}")
                nc.vector.tensor_tensor(tmp[:], qk_nd[:], cs, OP.mult)
                qk_r = qkrp.tile([128, 8, 4, D], BF16, tag="qkr",
                                 name=f"qkr{tp}")
                nc.vector.tensor_tensor(qk_r[:, :, :, 0:32],
                                        qk_nd[:, :, :, 32:64], sn0, OP.mult)
                nc.vector.tensor_tensor(qk_r[:, :, :, 0:32],
                                        tmp[:, :, :, 0:32],
                                        qk_r[:, :, :, 0:32], OP.subtract)
                yield
                nc.vector.tensor_tensor(qk_r[:, :, :, 32:64],
                                        qk_nd[:, :, :, 0:32], sn1, OP.mult)
                nc.vector.tensor_tensor(qk_r[:, :, :, 32:64],
                                        tmp[:, :, :, 32:64],
                                        qk_r[:, :, :, 32:64], OP.add)
                state[tp] = qk_r
                yield

            def transpose_gen(tp):
                """q,k -> [ch, {q,k}, n] via XBAR DMA transpose (bf16),
                freeing PE and DVE entirely."""
                qk_r = state.pop(tp)
                for j in range(8):
                    g = 8 * tp + j
                    col = 128 * g
                    nc.scalar.dma_start(
                        qkT[:, 0, col:col + 128],
                        qk_r[:, j, 0:2, :].rearrange("p a d -> p (a d)"),
                        transpose=True)
                    nc.scalar.dma_start(
                        qkT[:, 1, col:col + 128],
                        qk_r[:, j, 2:4, :].rearrange("p a d -> p (a d)"),
                        transpose=True)
                    if j == 3:
                        yield
                yield

            def attn_batch(b):
                """Attention for both local heads of batch b, head streams
                interleaved at 2-chunk-group granularity so each AV pair
                lands one group after its exp (no PE FIFO stall)."""
                col0 = N * b

                def s_exp(h, qt, g):
                    hof = D * h
                    qs = col0 + 512 * qt
                    psS = psSp.tile([128, 2, 512], F32, tag="pss",
                                    name=f"pS{b}{h}{qt}_{g}")
                    for j in range(2):
                        kc = 2 * g + j
                        nc.tensor.matmul(
                            psS[:, j, :],
                            qkT[hof:hof + D, 1,
                                col0 + 128 * kc:col0 + 128 * (kc + 1)],
                            qkT[hof:hof + D, 0, qs:qs + 512],
                            start=True, stop=True)
                    es = expp.tile([128, 2, 512], BF16, tag="es",
                                   name=f"es{b}{h}{qt}_{g}")
                    nc.scalar.activation(es[:], psS[:], AF.Exp, scale=0.125)
                    return es

                def av(h, qt, g, es, pav):
                    for j in range(2):
                        nc.tensor.matmul(
                            pav[:],
                            vext[:, 16 * b + 2 * g + j, h, 0:65],
                            es[:, j, :],
                            start=(g == 0 and j == 0),
                            stop=(g == 7 and j == 1))

                def norm(h, qt, pav):
                    hof = D * h
                    qs = col0 + 512 * qt
                    den = normp.tile([1, 512], BF16, tag="den",
                                     name=f"den{b}{h}{qt}")
                    nc.vector.tensor_copy(den[:], pav[64:65, :])
                    pbc = ps.tile([64, 512], F32, tag="ps",
                                  name=f"pbc{b}{h}{qt}")
                    nc.tensor.matmul(pbc[:], onesr[:], den[:],
                                     start=True, stop=True)
                    lnd = normp.tile([64, 512], F32, tag="lnd",
                                     name=f"lnd{b}{h}{qt}")
                    nc.scalar.activation(lnd[:], pbc[:], AF.Ln)
                    bcr = normp.tile([64, 512], F32, tag="bcr",
                                     name=f"bcr{b}{h}{qt}")
                    nc.scalar.activation(bcr[:], lnd[:], AF.Exp, scale=-1.0)
                    nc.vector.tensor_tensor(
                        attn_outT[hof:hof + D, qs:qs + 512],
                        pav[0:64, :], bcr[:], OP.mult)

                for qt in range(4):
                    pav = [psav.tile([65, 512], F32, tag="av",
                                     name=f"pav{b}{h}{qt}")
                           for h in range(2)]
                    pend = {}
                    for g in range(8):
                        for h in range(2):
                            es = s_exp(h, qt, g)
                            if h in pend:
                                av(h, qt, g - 1, pend[h], pav[h])
                            pend[h] = es
                    for h in range(2):
                        av(h, qt, 7, pend[h], pav[h])
                        norm(h, qt, pav[h])
                    yield

            # ---- AllToAll plumbing ---------------------------------------
            # batch 0: dest core j gets rows [j*256, (j+1)*256)         (one
            #   collective, fully hidden under batch-1 attention)
            # batch 1: two collectives so the first half of the projection
            #   overlaps the tail of attention:
            #   #2a: dest j rows [2048 + j*128, +128)      (ready after qt1)
            #   #2b: dest j rows [3072 + j*128, +128)      (ready after qt3)
            ccin0 = dram.tile([N_CORES, 128, HALF], BF16, name="ccin0")
            ccout0 = dram.tile([N_CORES, 128, HALF], BF16, name="ccout0")
            ccin1 = dram.tile([N_CORES, 128, HALF], BF16, name="ccin1")
            ccout1 = dram.tile([N_CORES, 128, HALF], BF16, name="ccout1")

            def emit_a2a0():
                for j in range(N_CORES):
                    c0 = HALF * j
                    nc.sync.dma_start(ccin0[j], attn_outT[:, c0:c0 + HALF])
                nc.gpsimd.collective_compute(
                    "AllToAll", OP.bypass,
                    replica_groups=[list(range(N_CORES))],
                    ins=[ccin0[:].opt()], outs=[ccout0[:].opt()])

            def emit_a2a1():
                for j in range(N_CORES):
                    c0 = N + HALF * j
                    nc.sync.dma_start(ccin1[j], attn_outT[:, c0:c0 + HALF])
                nc.gpsimd.collective_compute(
                    "AllToAll", OP.bypass,
                    replica_groups=[list(range(N_CORES))],
                    ins=[ccin1[:].opt()], outs=[ccout1[:].opt()])

            out_r = out_d.ap().rearrange("(t p) o -> p t o", p=128)

            def outproj_gen(nt):
                """project rows nt*128..+128 of this core's output."""
                gat = freqs.tile([128, 8, 128], BF16, tag="gat",
                                 name=f"gat{nt}")
                cc = ccout0 if nt < 2 else ccout1
                src_r = cc[:].rearrange("j p n -> p j n")
                nc.sync.dma_start(gat[:],
                                  src_r[:, :, 128 * (nt % 2):128 * (nt % 2 + 1)])
                ob = work.tile([128, C], F32, tag="ob", name=f"ob{nt}")
                for hf in range(2):
                    po = ps.tile([128, 512], F32, tag="ps",
                                 name=f"po{nt}_{hf}")
                    for cc in range(8):
                        nc.tensor.matmul(
                            po[:],
                            gat[:, cc, :],
                            wp_sb[:, cc, 512 * hf:512 * (hf + 1)],
                            start=(cc == 0), stop=(cc == 7))
                    nc.vector.tensor_tensor(
                        ob[:, 512 * hf:512 * (hf + 1)], po[:],
                        bias_sb[:, 512 * hf:512 * (hf + 1)], OP.add)
                    yield
                nc.sync.dma_start(out_r[:, nt, :], ob[:])
                yield

            def run_all(gen):
                for _ in gen:
                    pass

            def mix(main_gens, filler_gens, fill_per_step=2):
                fillers = list(filler_gens)
                for g in main_gens:
                    for _ in g:
                        took = 0
                        while fillers and took < fill_per_step:
                            try:
                                next(fillers[0])
                                took += 1
                            except StopIteration:
                                fillers.pop(0)
                for g in fillers:
                    run_all(g)

            # ---- emission schedule ---------------------------------------
            run_all(pre_gen(0))
            run_all(pre_gen(1))
            run_all(transpose_gen(0))
            run_all(transpose_gen(1))
            nc.sync.dma_start(wp_sb[:],
                              wpT_d.ap().rearrange("(co p) k -> p co k", p=128))
            mix([attn_batch(0)],
                [pre_gen(2), pre_gen(3), transpose_gen(2), transpose_gen(3)],
                fill_per_step=4)
            emit_a2a0()
            mix([attn_batch(1)],
                [outproj_gen(0), outproj_gen(1)], fill_per_step=2)
            emit_a2a1()
            run_all(outproj_gen(2))
            run_all(outproj_gen(3))
    _split_excess_waits(nc)
    return nc


_NC_CACHE = {}


def _get_nc():
    if "nc" not in _NC_CACHE:
        _NC_CACHE["nc"] = build()
    return _NC_CACHE["nc"]


def _prep_inputs(x, w_qkv, w_proj, b_proj, freqs_cos, freqs_sin):
    x = np.asarray(x, dtype=np.float32)
    w_qkv = np.asarray(w_qkv, dtype=np.float32)
    w_proj = np.asarray(w_proj, dtype=np.float32)
    b_proj = np.asarray(b_proj, dtype=np.float32)
    bf = ml_dtypes.bfloat16
    cos = np.asarray(freqs_cos, dtype=np.float32).reshape(N, D).astype(bf)
    sin = np.asarray(freqs_sin, dtype=np.float32).reshape(N, D).astype(bf)

    xT = np.ascontiguousarray(x.reshape(NTOT, C).T).astype(bf)
    wpT = np.ascontiguousarray(w_proj.T).astype(bf)
    biasb = np.ascontiguousarray(
        np.broadcast_to(b_proj, (128, C))).astype(np.float32)
    ident = np.eye(128, dtype=np.float32)

    in_maps = []
    for i in range(N_CORES):
        r0 = CPC * i
        wqkv = np.concatenate([w_qkv[r0:r0 + CPC],
                               w_qkv[C + r0:C + r0 + CPC],
                               w_qkv[2 * C + r0:2 * C + r0 + CPC]], axis=0)
        wqkvT = np.ascontiguousarray(wqkv.T).astype(bf)
        in_maps.append({
            "xT": xT, "wqkvT": wqkvT, "wpT": wpT,
            "biasb": biasb, "cosd": cos, "sind": sin, "identd": ident,
        })
    return in_maps


def kernel(x, w_qkv, w_proj, b_proj, freqs_cos, freqs_sin):
    in_maps = _prep_inputs(x, w_qkv, w_proj, b_proj, freqs_cos, freqs_sin)
    nc = _get_nc()
    res = run_bass_kernel_spmd(nc, in_maps, core_ids=list(range(N_CORES)))
    full = np.empty((NTOT, C), dtype=np.float32)
    for i in range(N_CORES):
        o = res.results[i]["out"]
        full[HALF * i:HALF * (i + 1)] = o[0:HALF]
        full[N + HALF * i:N + HALF * (i + 1)] = o[HALF:RPC]
    return full.reshape(B, N, C).astype(np.float32)


# revision 33
# speedup vs baseline: 1.2076x; 1.2076x over previous
"""Trainium2 8-core attention kernel (nn_Attention_19954418057485).

Sharding: heads are split across the 8 cores (2 heads = 128 channels
each); every core processes both batch elements for its heads.  Two
AllToAlls over all 8 cores (one per batch element, each overlapped with
compute) swap the channel axis for the row axis, so each core finishes
the full output projection for 512 rows (256 from each batch) of the
flattened (B*N, C) output.

Per-core pipeline (matmuls on PE in bf16, exp on ACT, elementwise DVE):
  x^T (bf16)  --PE-->  q,k (rows,ch) + v^T        [QKV projection]
  q,k: LayerNorm (d=64) + RoPE (bf16 DVE ops), then PE transposes to
  q^T,k^T [ch, n]; v^T -> V [n, ch] with a ones column appended.
  per (batch, head): S^T = K Q^T, exp(S/8) on ACT (no max-subtraction
  needed: layernormed q,k bound |scores| <= 8), AV accumulates
  V_ext^T @ expS^T giving out^T rows 0..63 plus the softmax denominator
  in row 64 (from the ones column).  Normalization: denominator row ->
  PE outer-product broadcast -> 1/x via ACT exp(-ln(x)) (same ACT table
  set as Exp; DVE reciprocal is 3.4us/op) -> one DVE multiply.

Instruction emission interleaves batch-0 attention (ACT-bound) with the
batch-1 preamble (DVE-bound) and batch-1 attention with the first half
of the output projection, keeping every engine's in-order queue busy.
"""
import sys

if "/opt/trn_rl_repo" not in sys.path:
    sys.path.insert(0, "/opt/trn_rl_repo")

import numpy as np
import ml_dtypes

import concourse.bass as bass
import concourse.tile as tile
from concourse import mybir
from concourse.bass_utils import run_bass_kernel_spmd

N_CORES = 8
B, N, C, H = 2, 2048, 1024, 16
D = 64
HPC = H // N_CORES          # heads per core = 2
CPC = HPC * D               # channels per core = 128
NTOT = B * N                # 4096 flattened rows
RPC = NTOT // N_CORES       # output rows per core = 512
HALF = RPC // 2             # rows per core per batch = 256
EPS = 1e-6

BF16 = mybir.dt.bfloat16
F32 = mybir.dt.float32
AF = mybir.ActivationFunctionType
OP = mybir.AluOpType
AX = mybir.AxisListType


def _split_excess_waits(nc, max_waits=1):
    """walrus rejects instructions with more than a couple of sem-wait
    commands; split extras onto preceding same-engine NoOps."""
    for fn in nc.m.functions:
        for blk in fn.blocks:
            new_insts = []
            for ins in blk.instructions:
                si = ins.sync_info
                ow = list(si.on_wait) if si is not None and si.on_wait else []
                if len(ow) > max_waits:
                    head = ow[: len(ow) - max_waits]
                    rest = ow[len(ow) - max_waits:]
                    for i in range(0, len(head), max_waits):
                        new_insts.append(mybir.InstNoOp(
                            name=f"{ins.name}_ws{i}",
                            engine=ins.engine,
                            ins=[], outs=[],
                            sync_info=mybir.SyncInfo(
                                on_wait=head[i:i + max_waits], on_update=[]),
                        ))
                    ins.sync_info = mybir.SyncInfo(
                        on_wait=rest, on_update=list(si.on_update or []))
                new_insts.append(ins)
            blk.instructions = new_insts


def build():
    nc = bass.Bass("TRN2", target_bir_lowering=False, debug=False,
                   num_devices=N_CORES)
    xT_d = nc.dram_tensor("xT", (C, NTOT), BF16, kind="ExternalInput")
    wqkv_d = nc.dram_tensor("wqkvT", (C, 3 * CPC), BF16, kind="ExternalInput")
    wpT_d = nc.dram_tensor("wpT", (C, C), BF16, kind="ExternalInput")
    bias_d = nc.dram_tensor("biasb", (128, C), F32, kind="ExternalInput")
    cos_d = nc.dram_tensor("cosd", (N, D), BF16, kind="ExternalInput")
    sin_d = nc.dram_tensor("sind", (N, D), BF16, kind="ExternalInput")
    ident_d = nc.dram_tensor("identd", (128, 128), F32, kind="ExternalInput")
    out_d = nc.dram_tensor("out", (RPC, C), F32, kind="ExternalOutput")

    with tile.TileContext(nc) as tc:
        with tc.tile_pool(name="consts", bufs=1) as consts, \
             tc.tile_pool(name="xload", bufs=2) as xload, \
             tc.tile_pool(name="qkrp", bufs=2) as qkrp, \
             tc.tile_pool(name="freqs", bufs=2) as freqs, \
             tc.tile_pool(name="work", bufs=3) as work, \
             tc.tile_pool(name="small", bufs=2) as small, \
             tc.tile_pool(name="exps", bufs=6) as expp, \
             tc.tile_pool(name="norm", bufs=2) as normp, \
             tc.tile_pool(name="ps", bufs=2, space="PSUM") as ps, \
             tc.tile_pool(name="psS", bufs=2, space="PSUM") as psSp, \
             tc.tile_pool(name="psav", bufs=2, space="PSUM") as psav, \
             tc.tile_pool(name="dram", bufs=1, space="DRAM") as dram:

            # ---- constants -------------------------------------------------
            wqkv_sb = consts.tile([128, 8, 3 * CPC], BF16)
            nc.sync.dma_start(wqkv_sb[:],
                              wqkv_d.ap().rearrange("(co p) k -> p co k", p=128))
            wp_sb = consts.tile([128, 8, C], BF16)      # DMA deferred
            bias_sb = consts.tile([128, C], F32)
            nc.sync.dma_start(bias_sb[:], bias_d.ap())
            cos_r = cos_d.ap().rearrange("(c p) d -> p c d", p=128)
            sin_r = sin_d.ap().rearrange("(c p) d -> p c d", p=128)
            ident_f = consts.tile([128, 128], F32)
            nc.sync.dma_start(ident_f[:], ident_d.ap())
            identr = consts.tile([128, 128], BF16)
            nc.scalar.activation(identr[:], ident_f[:], AF.Copy)
            onesr = consts.tile([1, 64], BF16)
            nc.scalar.activation(onesr[:], ident_f[0:1, 0:64], AF.Identity,
                                 scale=0.0, bias=1.0)

            # ---- persistent tensors ---------------------------------------
            qkT = consts.tile([128, 2, NTOT], BF16)   # [ch, {q,k}, b*N+n]
            # V with ones column: [n%128, chunk, head, 64 d + 1 one + pad]
            vext = consts.tile([128, 32, HPC, 66], BF16)
            nc.scalar.activation(
                vext[:, :, :, 64:65],
                bias_sb[:, 0:64].rearrange("p (a b c) -> p a b c", a=32, b=2),
                AF.Identity, scale=0.0, bias=1.0)
            attn_outT = consts.tile([128, NTOT], BF16)

            xT_r = xT_d.ap().rearrange("(co p) n -> p co n", p=128)
            state = {}

            def pre_gen(tp):
                """QKV proj + LN + RoPE for rows [tp*1024, (tp+1)*1024)."""
                xt = xload.tile([128, 8, 1024], BF16, tag="xt",
                                name=f"xt{tp}")
                nc.sync.dma_start(xt[:], xT_r[:, :, 1024 * tp:1024 * (tp + 1)])
                qk_nd = work.tile([128, 8, 4, D], BF16, tag="qknd",
                                  name=f"qknd{tp}")
                for ns in range(8):
                    pj = ps.tile([128, 3 * CPC], F32, tag="ps",
                                 name=f"pj{tp}_{ns}")
                    for cc in range(8):
                        nc.tensor.matmul(pj[:],
                                         xt[:, cc, 128 * ns:128 * (ns + 1)],
                                         wqkv_sb[:, cc, :],
                                         start=(cc == 0), stop=(cc == 7))
                    if tp == 0:
                        nc.scalar.activation(
                            qk_nd[:, ns],
                            pj[:, 0:2 * CPC].rearrange("p (s d) -> p s d",
                                                       s=4), AF.Copy)
                        nc.scalar.activation(
                            vext[:, 8 * tp + ns, :, 0:64],
                            pj[:, 2 * CPC:3 * CPC].rearrange(
                                "p (h d) -> p h d", h=HPC), AF.Copy)
                    else:
                        nc.vector.tensor_copy(
                            qk_nd[:, ns],
                            pj[:, 0:2 * CPC].rearrange("p (s d) -> p s d",
                                                       s=4))
                        nc.vector.tensor_copy(
                            vext[:, 8 * tp + ns, :, 0:64],
                            pj[:, 2 * CPC:3 * CPC].rearrange(
                                "p (h d) -> p h d", h=HPC))
                    if ns == 3:
                        yield
                yield
                # LayerNorm stats over d=64 for each (row, slot)
                s1 = small.tile([128, 8, 4], F32, tag="s1", name=f"s1_{tp}")
                nc.vector.reduce_sum(s1[:], qk_nd[:], axis=AX.X)
                sq = work.tile([128, 8, 4, D], BF16, tag="tmp",
                               name=f"sq{tp}")
                nc.scalar.square(sq[:], qk_nd[:])
                s2 = small.tile([128, 8, 4], F32, tag="s2", name=f"s2_{tp}")
                nc.vector.reduce_sum(s2[:], sq[:], axis=AX.X)
                mu = small.tile([128, 8, 4], F32, tag="mu", name=f"mu{tp}")
                nc.vector.tensor_scalar_mul(mu[:], s1[:], 1.0 / D)
                var = small.tile([128, 8, 4], F32, tag="var", name=f"var{tp}")
                nc.vector.tensor_scalar_mul(var[:], s2[:], 1.0 / D)
                mm = small.tile([128, 8, 4], F32, tag="mm", name=f"mm{tp}")
                nc.vector.tensor_tensor(mm[:], mu[:], mu[:], OP.mult)
                nc.vector.tensor_tensor(var[:], var[:], mm[:], OP.subtract)
                nc.vector.tensor_scalar_add(var[:], var[:], EPS)
                # rsqrt(var+eps) = exp(-0.5*ln(var+eps)) on ACT: stays in
                # the ln/exp table set (no thrash against attention's Exp)
                lnv = small.tile([128, 8, 4], F32, tag="lnv", name=f"lnv{tp}")
                nc.scalar.activation(lnv[:], var[:], AF.Ln)
                a_ = small.tile([128, 8, 4], BF16, tag="a", name=f"a{tp}")
                nc.scalar.activation(a_[:], lnv[:], AF.Exp, scale=-0.5)
                nma = small.tile([128, 8, 4], BF16, tag="nma", name=f"nma{tp}")
                nc.vector.tensor_tensor(nma[:], mu[:], a_[:], OP.mult)
                yield
                # qn = q*a - mu*a
                nc.vector.tensor_tensor(
                    qk_nd[:], qk_nd[:],
                    a_[:, :, :, None].to_broadcast((128, 8, 4, D)), OP.mult)
                nc.vector.tensor_tensor(
                    qk_nd[:], qk_nd[:],
                    nma[:, :, :, None].to_broadcast((128, 8, 4, D)),
                    OP.subtract)
                yield
                # RoPE: out = qn*cos + rot_half(qn)*sin
                cs_lo = 8 * (tp % 2)
                cos_t = freqs.tile([128, 8, D], BF16, tag="cos",
                                   name=f"cos{tp}")
                nc.sync.dma_start(cos_t[:], cos_r[:, cs_lo:cs_lo + 8, :])
                sin_t = freqs.tile([128, 8, D], BF16, tag="sin",
                                   name=f"sin{tp}")
                nc.sync.dma_start(sin_t[:], sin_r[:, cs_lo:cs_lo + 8, :])
                cs = cos_t[:, :, None, :].to_broadcast((128, 8, 4, D))
                sn0 = sin_t[:, :, None, 0:32].to_broadcast((128, 8, 4, 32))
                sn1 = sin_t[:, :, None, 32:64].to_broadcast((128, 8, 4, 32))
                tmp = work.tile([128, 8, 4, D], BF16, tag="tmp",
                                name=f"tmp{tp}")
                nc.vector.tensor_tensor(tmp[:], qk_nd[:], cs, OP.mult)
                qk_r = qkrp.tile([128, 8, 4, D], BF16, tag="qkr",
                                 name=f"qkr{tp}")
                nc.vector.tensor_tensor(qk_r[:, :, :, 0:32],
                                        qk_nd[:, :, :, 32:64], sn0, OP.mult)
                nc.vector.tensor_tensor(qk_r[:, :, :, 0:32],
                                        tmp[:, :, :, 0:32],
                                        qk_r[:, :, :, 0:32], OP.subtract)
                yield
                nc.vector.tensor_tensor(qk_r[:, :, :, 32:64],
                                        qk_nd[:, :, :, 0:32], sn1, OP.mult)
                nc.vector.tensor_tensor(qk_r[:, :, :, 32:64],
                                        tmp[:, :, :, 32:64],
                                        qk_r[:, :, :, 32:64], OP.add)
                state[tp] = qk_r
                yield

            def transpose_gen(tp):
                """PE transposes: q,k -> [ch, {q,k}, n]."""
                qk_r = state.pop(tp)
                for j in range(8):
                    g = 8 * tp + j
                    col = 128 * g
                    ptqk = ps.tile([128, 2, 128], BF16, tag="ps",
                                   name=f"ptqk{g}")
                    nc.tensor.transpose(ptqk[:, 0, :], qk_r[:, j, 0:2, :],
                                        identr[:])
                    nc.tensor.transpose(ptqk[:, 1, :], qk_r[:, j, 2:4, :],
                                        identr[:])
                    nc.vector.tensor_copy(qkT[:, :, col:col + 128], ptqk[:])
                    if j == 3:
                        yield
                yield

            def attn_batch(b):
                """Attention for both local heads of batch b, head streams
                interleaved at 2-chunk-group granularity so each AV pair
                lands one group after its exp (no PE FIFO stall)."""
                col0 = N * b

                def s_exp(h, qt, g):
                    hof = D * h
                    qs = col0 + 512 * qt
                    psS = psSp.tile([128, 2, 512], F32, tag="pss",
                                    name=f"pS{b}{h}{qt}_{g}")
                    for j in range(2):
                        kc = 2 * g + j
                        nc.tensor.matmul(
                            psS[:, j, :],
                            qkT[hof:hof + D, 1,
                                col0 + 128 * kc:col0 + 128 * (kc + 1)],
                            qkT[hof:hof + D, 0, qs:qs + 512],
                            start=True, stop=True)
                    es = expp.tile([128, 2, 512], BF16, tag="es",
                                   name=f"es{b}{h}{qt}_{g}")
                    nc.scalar.activation(es[:], psS[:], AF.Exp, scale=0.125)
                    return es

                def av(h, qt, g, es, pav):
                    for j in range(2):
                        nc.tensor.matmul(
                            pav[:],
                            vext[:, 16 * b + 2 * g + j, h, 0:65],
                            es[:, j, :],
                            start=(g == 0 and j == 0),
                            stop=(g == 7 and j == 1))

                def norm(h, qt, pav):
                    hof = D * h
                    qs = col0 + 512 * qt
                    den = normp.tile([1, 512], BF16, tag="den",
                                     name=f"den{b}{h}{qt}")
                    nc.vector.tensor_copy(den[:], pav[64:65, :])
                    pbc = ps.tile([64, 512], F32, tag="ps",
                                  name=f"pbc{b}{h}{qt}")
                    nc.tensor.matmul(pbc[:], onesr[:], den[:],
                                     start=True, stop=True)
                    lnd = normp.tile([64, 512], F32, tag="lnd",
                                     name=f"lnd{b}{h}{qt}")
                    nc.scalar.activation(lnd[:], pbc[:], AF.Ln)
                    bcr = normp.tile([64, 512], F32, tag="bcr",
                                     name=f"bcr{b}{h}{qt}")
                    nc.scalar.activation(bcr[:], lnd[:], AF.Exp, scale=-1.0)
                    nc.vector.tensor_tensor(
                        attn_outT[hof:hof + D, qs:qs + 512],
                        pav[0:64, :], bcr[:], OP.mult)

                for qt in range(4):
                    pav = [psav.tile([65, 512], F32, tag="av",
                                     name=f"pav{b}{h}{qt}")
                           for h in range(2)]
                    pend = {}
                    for g in range(8):
                        for h in range(2):
                            es = s_exp(h, qt, g)
                            if h in pend:
                                av(h, qt, g - 1, pend[h], pav[h])
                            pend[h] = es
                    for h in range(2):
                        av(h, qt, 7, pend[h], pav[h])
                        norm(h, qt, pav[h])
                    yield

            # ---- AllToAll plumbing ---------------------------------------
            # batch 0: dest core j gets rows [j*256, (j+1)*256)         (one
            #   collective, fully hidden under batch-1 attention)
            # batch 1: two collectives so the first half of the projection
            #   overlaps the tail of attention:
            #   #2a: dest j rows [2048 + j*128, +128)      (ready after qt1)
            #   #2b: dest j rows [3072 + j*128, +128)      (ready after qt3)
            ccin0 = dram.tile([N_CORES, 128, HALF], BF16, name="ccin0")
            ccout0 = dram.tile([N_CORES, 128, HALF], BF16, name="ccout0")
            ccin1 = dram.tile([N_CORES, 128, HALF], BF16, name="ccin1")
            ccout1 = dram.tile([N_CORES, 128, HALF], BF16, name="ccout1")

            def emit_a2a0():
                for j in range(N_CORES):
                    c0 = HALF * j
                    nc.sync.dma_start(ccin0[j], attn_outT[:, c0:c0 + HALF])
                nc.gpsimd.collective_compute(
                    "AllToAll", OP.bypass,
                    replica_groups=[list(range(N_CORES))],
                    ins=[ccin0[:].opt()], outs=[ccout0[:].opt()])

            def emit_a2a1():
                for j in range(N_CORES):
                    c0 = N + HALF * j
                    nc.sync.dma_start(ccin1[j], attn_outT[:, c0:c0 + HALF])
                nc.gpsimd.collective_compute(
                    "AllToAll", OP.bypass,
                    replica_groups=[list(range(N_CORES))],
                    ins=[ccin1[:].opt()], outs=[ccout1[:].opt()])

            out_r = out_d.ap().rearrange("(t p) o -> p t o", p=128)

            def outproj_gen(nt):
                """project rows nt*128..+128 of this core's output."""
                gat = freqs.tile([128, 8, 128], BF16, tag="gat",
                                 name=f"gat{nt}")
                cc = ccout0 if nt < 2 else ccout1
                src_r = cc[:].rearrange("j p n -> p j n")
                nc.sync.dma_start(gat[:],
                                  src_r[:, :, 128 * (nt % 2):128 * (nt % 2 + 1)])
                ob = work.tile([128, C], F32, tag="ob", name=f"ob{nt}")
                for hf in range(2):
                    po = ps.tile([128, 512], F32, tag="ps",
                                 name=f"po{nt}_{hf}")
                    for cc in range(8):
                        nc.tensor.matmul(
                            po[:],
                            gat[:, cc, :],
                            wp_sb[:, cc, 512 * hf:512 * (hf + 1)],
                            start=(cc == 0), stop=(cc == 7))
                    nc.vector.tensor_tensor(
                        ob[:, 512 * hf:512 * (hf + 1)], po[:],
                        bias_sb[:, 512 * hf:512 * (hf + 1)], OP.add)
                    yield
                nc.sync.dma_start(out_r[:, nt, :], ob[:])
                yield

            def run_all(gen):
                for _ in gen:
                    pass

            def mix(main_gens, filler_gens, fill_per_step=2):
                fillers = list(filler_gens)
                for g in main_gens:
                    for _ in g:
                        took = 0
                        while fillers and took < fill_per_step:
                            try:
                                next(fillers[0])
                                took += 1
                            except StopIteration:
                                fillers.pop(0)
                for g in fillers:
                    run_all(g)

            # ---- emission schedule ---------------------------------------
            run_all(pre_gen(0))
            run_all(pre_gen(1))
            run_all(transpose_gen(0))
            run_all(transpose_gen(1))
            nc.sync.dma_start(wp_sb[:],
                              wpT_d.ap().rearrange("(co p) k -> p co k", p=128))
            mix([attn_batch(0)],
                [pre_gen(2), pre_gen(3), transpose_gen(2), transpose_gen(3)],
                fill_per_step=4)
            emit_a2a0()
            mix([attn_batch(1)],
                [outproj_gen(0), outproj_gen(1)], fill_per_step=2)
            emit_a2a1()
            run_all(outproj_gen(2))
            run_all(outproj_gen(3))
    _split_excess_waits(nc)
    return nc


_NC_CACHE = {}


def _get_nc():
    if "nc" not in _NC_CACHE:
        _NC_CACHE["nc"] = build()
    return _NC_CACHE["nc"]


def _prep_inputs(x, w_qkv, w_proj, b_proj, freqs_cos, freqs_sin):
    x = np.asarray(x, dtype=np.float32)
    w_qkv = np.asarray(w_qkv, dtype=np.float32)
    w_proj = np.asarray(w_proj, dtype=np.float32)
    b_proj = np.asarray(b_proj, dtype=np.float32)
    bf = ml_dtypes.bfloat16
    cos = np.asarray(freqs_cos, dtype=np.float32).reshape(N, D).astype(bf)
    sin = np.asarray(freqs_sin, dtype=np.float32).reshape(N, D).astype(bf)

    xT = np.ascontiguousarray(x.reshape(NTOT, C).T).astype(bf)
    wpT = np.ascontiguousarray(w_proj.T).astype(bf)
    biasb = np.ascontiguousarray(
        np.broadcast_to(b_proj, (128, C))).astype(np.float32)
    ident = np.eye(128, dtype=np.float32)

    in_maps = []
    for i in range(N_CORES):
        r0 = CPC * i
        wqkv = np.concatenate([w_qkv[r0:r0 + CPC],
                               w_qkv[C + r0:C + r0 + CPC],
                               w_qkv[2 * C + r0:2 * C + r0 + CPC]], axis=0)
        wqkvT = np.ascontiguousarray(wqkv.T).astype(bf)
        in_maps.append({
            "xT": xT, "wqkvT": wqkvT, "wpT": wpT,
            "biasb": biasb, "cosd": cos, "sind": sin, "identd": ident,
        })
    return in_maps


def kernel(x, w_qkv, w_proj, b_proj, freqs_cos, freqs_sin):
    in_maps = _prep_inputs(x, w_qkv, w_proj, b_proj, freqs_cos, freqs_sin)
    nc = _get_nc()
    res = run_bass_kernel_spmd(nc, in_maps, core_ids=list(range(N_CORES)))
    full = np.empty((NTOT, C), dtype=np.float32)
    for i in range(N_CORES):
        o = res.results[i]["out"]
        full[HALF * i:HALF * (i + 1)] = o[0:HALF]
        full[N + HALF * i:N + HALF * (i + 1)] = o[HALF:RPC]
    return full.reshape(B, N, C).astype(np.float32)


# revision 34
# speedup vs baseline: 1.2141x; 1.0054x over previous
"""Trainium2 8-core attention kernel (nn_Attention_19954418057485).

Sharding: heads are split across the 8 cores (2 heads = 128 channels
each); every core processes both batch elements for its heads.  Two
AllToAlls over all 8 cores (one per batch element, each overlapped with
compute) swap the channel axis for the row axis, so each core finishes
the full output projection for 512 rows (256 from each batch) of the
flattened (B*N, C) output.

Per-core pipeline (matmuls on PE in bf16, exp on ACT, elementwise DVE):
  x^T (bf16)  --PE-->  q,k (rows,ch) + v^T        [QKV projection]
  q,k: LayerNorm (d=64) + RoPE (bf16 DVE ops), then PE transposes to
  q^T,k^T [ch, n]; v^T -> V [n, ch] with a ones column appended.
  per (batch, head): S^T = K Q^T, exp(S/8) on ACT (no max-subtraction
  needed: layernormed q,k bound |scores| <= 8), AV accumulates
  V_ext^T @ expS^T giving out^T rows 0..63 plus the softmax denominator
  in row 64 (from the ones column).  Normalization: denominator row ->
  PE outer-product broadcast -> 1/x via ACT exp(-ln(x)) (same ACT table
  set as Exp; DVE reciprocal is 3.4us/op) -> one DVE multiply.

Instruction emission interleaves batch-0 attention (ACT-bound) with the
batch-1 preamble (DVE-bound) and batch-1 attention with the first half
of the output projection, keeping every engine's in-order queue busy.
"""
import sys

if "/opt/trn_rl_repo" not in sys.path:
    sys.path.insert(0, "/opt/trn_rl_repo")

import numpy as np
import ml_dtypes

import concourse.bass as bass
import concourse.tile as tile
from concourse import mybir
from concourse.bass_utils import run_bass_kernel_spmd

N_CORES = 8
B, N, C, H = 2, 2048, 1024, 16
D = 64
HPC = H // N_CORES          # heads per core = 2
CPC = HPC * D               # channels per core = 128
NTOT = B * N                # 4096 flattened rows
RPC = NTOT // N_CORES       # output rows per core = 512
HALF = RPC // 2             # rows per core per batch = 256
EPS = 1e-6

BF16 = mybir.dt.bfloat16
F32 = mybir.dt.float32
AF = mybir.ActivationFunctionType
OP = mybir.AluOpType
AX = mybir.AxisListType


def _split_excess_waits(nc, max_waits=1):
    """walrus rejects instructions with more than a couple of sem-wait
    commands; split extras onto preceding same-engine NoOps."""
    for fn in nc.m.functions:
        for blk in fn.blocks:
            new_insts = []
            for ins in blk.instructions:
                si = ins.sync_info
                ow = list(si.on_wait) if si is not None and si.on_wait else []
                if len(ow) > max_waits:
                    head = ow[: len(ow) - max_waits]
                    rest = ow[len(ow) - max_waits:]
                    for i in range(0, len(head), max_waits):
                        new_insts.append(mybir.InstNoOp(
                            name=f"{ins.name}_ws{i}",
                            engine=ins.engine,
                            ins=[], outs=[],
                            sync_info=mybir.SyncInfo(
                                on_wait=head[i:i + max_waits], on_update=[]),
                        ))
                    ins.sync_info = mybir.SyncInfo(
                        on_wait=rest, on_update=list(si.on_update or []))
                new_insts.append(ins)
            blk.instructions = new_insts


def build():
    nc = bass.Bass("TRN2", target_bir_lowering=False, debug=False,
                   num_devices=N_CORES)
    xT_d = nc.dram_tensor("xT", (C, NTOT), BF16, kind="ExternalInput")
    wqkv_d = nc.dram_tensor("wqkvT", (C, 3 * CPC), BF16, kind="ExternalInput")
    wpT_d = nc.dram_tensor("wpT", (C, C), BF16, kind="ExternalInput")
    bias_d = nc.dram_tensor("biasb", (128, C), F32, kind="ExternalInput")
    cos_d = nc.dram_tensor("cosd", (N, D), BF16, kind="ExternalInput")
    sin_d = nc.dram_tensor("sind", (N, D), BF16, kind="ExternalInput")
    ident_d = nc.dram_tensor("identd", (128, 128), F32, kind="ExternalInput")
    out_d = nc.dram_tensor("out", (RPC, C), F32, kind="ExternalOutput")

    with tile.TileContext(nc) as tc:
        with tc.tile_pool(name="consts", bufs=1) as consts, \
             tc.tile_pool(name="xload", bufs=2) as xload, \
             tc.tile_pool(name="qkrp", bufs=2) as qkrp, \
             tc.tile_pool(name="freqs", bufs=2) as freqs, \
             tc.tile_pool(name="work", bufs=3) as work, \
             tc.tile_pool(name="small", bufs=2) as small, \
             tc.tile_pool(name="exps", bufs=6) as expp, \
             tc.tile_pool(name="norm", bufs=2) as normp, \
             tc.tile_pool(name="ps", bufs=2, space="PSUM") as ps, \
             tc.tile_pool(name="psS", bufs=2, space="PSUM") as psSp, \
             tc.tile_pool(name="psav", bufs=2, space="PSUM") as psav, \
             tc.tile_pool(name="dram", bufs=1, space="DRAM") as dram:

            # ---- constants -------------------------------------------------
            wqkv_sb = consts.tile([128, 8, 3 * CPC], BF16)
            nc.sync.dma_start(wqkv_sb[:],
                              wqkv_d.ap().rearrange("(co p) k -> p co k", p=128))
            wp_sb = consts.tile([128, 8, C], BF16)      # DMA deferred
            bias_sb = consts.tile([128, C], F32)
            nc.sync.dma_start(bias_sb[:], bias_d.ap())
            cos_r = cos_d.ap().rearrange("(c p) d -> p c d", p=128)
            sin_r = sin_d.ap().rearrange("(c p) d -> p c d", p=128)
            ident_f = consts.tile([128, 128], F32)
            nc.sync.dma_start(ident_f[:], ident_d.ap())
            identr = consts.tile([128, 128], BF16)
            nc.scalar.activation(identr[:], ident_f[:], AF.Copy)
            onesr = consts.tile([1, 64], BF16)
            nc.scalar.activation(onesr[:], ident_f[0:1, 0:64], AF.Identity,
                                 scale=0.0, bias=1.0)

            # ---- persistent tensors ---------------------------------------
            qkT = consts.tile([128, 2, NTOT], BF16)   # [ch, {q,k}, b*N+n]
            # V with ones column: [n%128, chunk, head, 64 d + 1 one + pad]
            vext = consts.tile([128, 32, HPC, 66], BF16)
            nc.scalar.activation(
                vext[:, :, :, 64:65],
                bias_sb[:, 0:64].rearrange("p (a b c) -> p a b c", a=32, b=2),
                AF.Identity, scale=0.0, bias=1.0)
            attn_outT = consts.tile([128, NTOT], BF16)

            xT_r = xT_d.ap().rearrange("(co p) n -> p co n", p=128)
            state = {}

            def pre_gen(tp):
                """QKV proj + LN + RoPE for rows [tp*1024, (tp+1)*1024)."""
                xt = xload.tile([128, 8, 1024], BF16, tag="xt",
                                name=f"xt{tp}")
                nc.sync.dma_start(xt[:], xT_r[:, :, 1024 * tp:1024 * (tp + 1)])
                qk_nd = work.tile([128, 8, 4, D], BF16, tag="qknd",
                                  name=f"qknd{tp}")
                for ns in range(8):
                    pj = ps.tile([128, 3 * CPC], F32, tag="ps",
                                 name=f"pj{tp}_{ns}")
                    for cc in range(8):
                        nc.tensor.matmul(pj[:],
                                         xt[:, cc, 128 * ns:128 * (ns + 1)],
                                         wqkv_sb[:, cc, :],
                                         start=(cc == 0), stop=(cc == 7))
                    if tp <= 1:
                        nc.scalar.activation(
                            qk_nd[:, ns],
                            pj[:, 0:2 * CPC].rearrange("p (s d) -> p s d",
                                                       s=4), AF.Copy)
                        nc.scalar.activation(
                            vext[:, 8 * tp + ns, :, 0:64],
                            pj[:, 2 * CPC:3 * CPC].rearrange(
                                "p (h d) -> p h d", h=HPC), AF.Copy)
                    else:
                        nc.vector.tensor_copy(
                            qk_nd[:, ns],
                            pj[:, 0:2 * CPC].rearrange("p (s d) -> p s d",
                                                       s=4))
                        nc.vector.tensor_copy(
                            vext[:, 8 * tp + ns, :, 0:64],
                            pj[:, 2 * CPC:3 * CPC].rearrange(
                                "p (h d) -> p h d", h=HPC))
                    if ns == 3:
                        yield
                yield
                # LayerNorm stats over d=64 for each (row, slot)
                s1 = small.tile([128, 8, 4], F32, tag="s1", name=f"s1_{tp}")
                nc.vector.reduce_sum(s1[:], qk_nd[:], axis=AX.X)
                sq = work.tile([128, 8, 4, D], BF16, tag="tmp",
                               name=f"sq{tp}")
                nc.scalar.square(sq[:], qk_nd[:])
                s2 = small.tile([128, 8, 4], F32, tag="s2", name=f"s2_{tp}")
                nc.vector.reduce_sum(s2[:], sq[:], axis=AX.X)
                mu = small.tile([128, 8, 4], F32, tag="mu", name=f"mu{tp}")
                nc.vector.tensor_scalar_mul(mu[:], s1[:], 1.0 / D)
                var = small.tile([128, 8, 4], F32, tag="var", name=f"var{tp}")
                nc.vector.tensor_scalar_mul(var[:], s2[:], 1.0 / D)
                mm = small.tile([128, 8, 4], F32, tag="mm", name=f"mm{tp}")
                nc.vector.tensor_tensor(mm[:], mu[:], mu[:], OP.mult)
                nc.vector.tensor_tensor(var[:], var[:], mm[:], OP.subtract)
                nc.vector.tensor_scalar_add(var[:], var[:], EPS)
                # rsqrt(var+eps) = exp(-0.5*ln(var+eps)) on ACT: stays in
                # the ln/exp table set (no thrash against attention's Exp)
                lnv = small.tile([128, 8, 4], F32, tag="lnv", name=f"lnv{tp}")
                nc.scalar.activation(lnv[:], var[:], AF.Ln)
                a_ = small.tile([128, 8, 4], BF16, tag="a", name=f"a{tp}")
                nc.scalar.activation(a_[:], lnv[:], AF.Exp, scale=-0.5)
                nma = small.tile([128, 8, 4], BF16, tag="nma", name=f"nma{tp}")
                nc.vector.tensor_tensor(nma[:], mu[:], a_[:], OP.mult)
                yield
                # qn = q*a - mu*a
                nc.vector.tensor_tensor(
                    qk_nd[:], qk_nd[:],
                    a_[:, :, :, None].to_broadcast((128, 8, 4, D)), OP.mult)
                nc.vector.tensor_tensor(
                    qk_nd[:], qk_nd[:],
                    nma[:, :, :, None].to_broadcast((128, 8, 4, D)),
                    OP.subtract)
                yield
                # RoPE: out = qn*cos + rot_half(qn)*sin
                cs_lo = 8 * (tp % 2)
                cos_t = freqs.tile([128, 8, D], BF16, tag="cos",
                                   name=f"cos{tp}")
                nc.sync.dma_start(cos_t[:], cos_r[:, cs_lo:cs_lo + 8, :])
                sin_t = freqs.tile([128, 8, D], BF16, tag="sin",
                                   name=f"sin{tp}")
                nc.sync.dma_start(sin_t[:], sin_r[:, cs_lo:cs_lo + 8, :])
                cs = cos_t[:, :, None, :].to_broadcast((128, 8, 4, D))
                sn0 = sin_t[:, :, None, 0:32].to_broadcast((128, 8, 4, 32))
                sn1 = sin_t[:, :, None, 32:64].to_broadcast((128, 8, 4, 32))
                tmp = work.tile([128, 8, 4, D], BF16, tag="tmp",
                                name=f"tmp{tp}")
                nc.vector.tensor_tensor(tmp[:], qk_nd[:], cs, OP.mult)
                qk_r = qkrp.tile([128, 8, 4, D], BF16, tag="qkr",
                                 name=f"qkr{tp}")
                nc.vector.tensor_tensor(qk_r[:, :, :, 0:32],
                                        qk_nd[:, :, :, 32:64], sn0, OP.mult)
                nc.vector.tensor_tensor(qk_r[:, :, :, 0:32],
                                        tmp[:, :, :, 0:32],
                                        qk_r[:, :, :, 0:32], OP.subtract)
                yield
                nc.vector.tensor_tensor(qk_r[:, :, :, 32:64],
                                        qk_nd[:, :, :, 0:32], sn1, OP.mult)
                nc.vector.tensor_tensor(qk_r[:, :, :, 32:64],
                                        tmp[:, :, :, 32:64],
                                        qk_r[:, :, :, 32:64], OP.add)
                state[tp] = qk_r
                yield

            def transpose_gen(tp):
                """PE transposes: q,k -> [ch, {q,k}, n]."""
                qk_r = state.pop(tp)
                for j in range(8):
                    g = 8 * tp + j
                    col = 128 * g
                    ptqk = ps.tile([128, 2, 128], BF16, tag="ps",
                                   name=f"ptqk{g}")
                    nc.tensor.transpose(ptqk[:, 0, :], qk_r[:, j, 0:2, :],
                                        identr[:])
                    nc.tensor.transpose(ptqk[:, 1, :], qk_r[:, j, 2:4, :],
                                        identr[:])
                    nc.vector.tensor_copy(qkT[:, :, col:col + 128], ptqk[:])
                    if j == 3:
                        yield
                yield

            def attn_batch(b):
                """Attention for both local heads of batch b, head streams
                interleaved at 2-chunk-group granularity so each AV pair
                lands one group after its exp (no PE FIFO stall)."""
                col0 = N * b

                def s_exp(h, qt, g):
                    hof = D * h
                    qs = col0 + 512 * qt
                    psS = psSp.tile([128, 2, 512], F32, tag="pss",
                                    name=f"pS{b}{h}{qt}_{g}")
                    for j in range(2):
                        kc = 2 * g + j
                        nc.tensor.matmul(
                            psS[:, j, :],
                            qkT[hof:hof + D, 1,
                                col0 + 128 * kc:col0 + 128 * (kc + 1)],
                            qkT[hof:hof + D, 0, qs:qs + 512],
                            start=True, stop=True)
                    es = expp.tile([128, 2, 512], BF16, tag="es",
                                   name=f"es{b}{h}{qt}_{g}")
                    nc.scalar.activation(es[:], psS[:], AF.Exp, scale=0.125)
                    return es

                def av(h, qt, g, es, pav):
                    for j in range(2):
                        nc.tensor.matmul(
                            pav[:],
                            vext[:, 16 * b + 2 * g + j, h, 0:65],
                            es[:, j, :],
                            start=(g == 0 and j == 0),
                            stop=(g == 7 and j == 1))

                def norm(h, qt, pav):
                    hof = D * h
                    qs = col0 + 512 * qt
                    den = normp.tile([1, 512], BF16, tag="den",
                                     name=f"den{b}{h}{qt}")
                    nc.vector.tensor_copy(den[:], pav[64:65, :])
                    pbc = ps.tile([64, 512], F32, tag="ps",
                                  name=f"pbc{b}{h}{qt}")
                    nc.tensor.matmul(pbc[:], onesr[:], den[:],
                                     start=True, stop=True)
                    lnd = normp.tile([64, 512], F32, tag="lnd",
                                     name=f"lnd{b}{h}{qt}")
                    nc.scalar.activation(lnd[:], pbc[:], AF.Ln)
                    bcr = normp.tile([64, 512], F32, tag="bcr",
                                     name=f"bcr{b}{h}{qt}")
                    nc.scalar.activation(bcr[:], lnd[:], AF.Exp, scale=-1.0)
                    nc.vector.tensor_tensor(
                        attn_outT[hof:hof + D, qs:qs + 512],
                        pav[0:64, :], bcr[:], OP.mult)

                for qt in range(4):
                    pav = [psav.tile([65, 512], F32, tag="av",
                                     name=f"pav{b}{h}{qt}")
                           for h in range(2)]
                    pend = {}
                    for g in range(8):
                        for h in range(2):
                            es = s_exp(h, qt, g)
                            if h in pend:
                                av(h, qt, g - 1, pend[h], pav[h])
                            pend[h] = es
                    for h in range(2):
                        av(h, qt, 7, pend[h], pav[h])
                        norm(h, qt, pav[h])
                    yield

            # ---- AllToAll plumbing ---------------------------------------
            # batch 0: dest core j gets rows [j*256, (j+1)*256)         (one
            #   collective, fully hidden under batch-1 attention)
            # batch 1: two collectives so the first half of the projection
            #   overlaps the tail of attention:
            #   #2a: dest j rows [2048 + j*128, +128)      (ready after qt1)
            #   #2b: dest j rows [3072 + j*128, +128)      (ready after qt3)
            ccin0 = dram.tile([N_CORES, 128, HALF], BF16, name="ccin0")
            ccout0 = dram.tile([N_CORES, 128, HALF], BF16, name="ccout0")
            ccin1 = dram.tile([N_CORES, 128, HALF], BF16, name="ccin1")
            ccout1 = dram.tile([N_CORES, 128, HALF], BF16, name="ccout1")

            def emit_a2a0():
                for j in range(N_CORES):
                    c0 = HALF * j
                    nc.sync.dma_start(ccin0[j], attn_outT[:, c0:c0 + HALF])
                nc.gpsimd.collective_compute(
                    "AllToAll", OP.bypass,
                    replica_groups=[list(range(N_CORES))],
                    ins=[ccin0[:].opt()], outs=[ccout0[:].opt()])

            def emit_a2a1():
                for j in range(N_CORES):
                    c0 = N + HALF * j
                    nc.sync.dma_start(ccin1[j], attn_outT[:, c0:c0 + HALF])
                nc.gpsimd.collective_compute(
                    "AllToAll", OP.bypass,
                    replica_groups=[list(range(N_CORES))],
                    ins=[ccin1[:].opt()], outs=[ccout1[:].opt()])

            out_r = out_d.ap().rearrange("(t p) o -> p t o", p=128)

            def outproj_gen(nt):
                """project rows nt*128..+128 of this core's output."""
                gat = freqs.tile([128, 8, 128], BF16, tag="gat",
                                 name=f"gat{nt}")
                cc = ccout0 if nt < 2 else ccout1
                src_r = cc[:].rearrange("j p n -> p j n")
                nc.sync.dma_start(gat[:],
                                  src_r[:, :, 128 * (nt % 2):128 * (nt % 2 + 1)])
                ob = work.tile([128, C], F32, tag="ob", name=f"ob{nt}")
                for hf in range(2):
                    po = ps.tile([128, 512], F32, tag="ps",
                                 name=f"po{nt}_{hf}")
                    for cc in range(8):
                        nc.tensor.matmul(
                            po[:],
                            gat[:, cc, :],
                            wp_sb[:, cc, 512 * hf:512 * (hf + 1)],
                            start=(cc == 0), stop=(cc == 7))
                    nc.vector.tensor_tensor(
                        ob[:, 512 * hf:512 * (hf + 1)], po[:],
                        bias_sb[:, 512 * hf:512 * (hf + 1)], OP.add)
                    yield
                nc.sync.dma_start(out_r[:, nt, :], ob[:])
                yield

            def run_all(gen):
                for _ in gen:
                    pass

            def mix(main_gens, filler_gens, fill_per_step=2):
                fillers = list(filler_gens)
                for g in main_gens:
                    for _ in g:
                        took = 0
                        while fillers and took < fill_per_step:
                            try:
                                next(fillers[0])
                                took += 1
                            except StopIteration:
                                fillers.pop(0)
                for g in fillers:
                    run_all(g)

            # ---- emission schedule ---------------------------------------
            run_all(pre_gen(0))
            run_all(pre_gen(1))
            run_all(transpose_gen(0))
            run_all(transpose_gen(1))
            nc.sync.dma_start(wp_sb[:],
                              wpT_d.ap().rearrange("(co p) k -> p co k", p=128))
            mix([attn_batch(0)],
                [pre_gen(2), pre_gen(3), transpose_gen(2), transpose_gen(3)],
                fill_per_step=4)
            emit_a2a0()
            mix([attn_batch(1)],
                [outproj_gen(0), outproj_gen(1)], fill_per_step=2)
            emit_a2a1()
            run_all(outproj_gen(2))
            run_all(outproj_gen(3))
    _split_excess_waits(nc)
    return nc


_NC_CACHE = {}


def _get_nc():
    if "nc" not in _NC_CACHE:
        _NC_CACHE["nc"] = build()
    return _NC_CACHE["nc"]


def _prep_inputs(x, w_qkv, w_proj, b_proj, freqs_cos, freqs_sin):
    x = np.asarray(x, dtype=np.float32)
    w_qkv = np.asarray(w_qkv, dtype=np.float32)
    w_proj = np.asarray(w_proj, dtype=np.float32)
    b_proj = np.asarray(b_proj, dtype=np.float32)
    bf = ml_dtypes.bfloat16
    cos = np.asarray(freqs_cos, dtype=np.float32).reshape(N, D).astype(bf)
    sin = np.asarray(freqs_sin, dtype=np.float32).reshape(N, D).astype(bf)

    xT = np.ascontiguousarray(x.reshape(NTOT, C).T).astype(bf)
    wpT = np.ascontiguousarray(w_proj.T).astype(bf)
    biasb = np.ascontiguousarray(
        np.broadcast_to(b_proj, (128, C))).astype(np.float32)
    ident = np.eye(128, dtype=np.float32)

    in_maps = []
    for i in range(N_CORES):
        r0 = CPC * i
        wqkv = np.concatenate([w_qkv[r0:r0 + CPC],
                               w_qkv[C + r0:C + r0 + CPC],
                               w_qkv[2 * C + r0:2 * C + r0 + CPC]], axis=0)
        wqkvT = np.ascontiguousarray(wqkv.T).astype(bf)
        in_maps.append({
            "xT": xT, "wqkvT": wqkvT, "wpT": wpT,
            "biasb": biasb, "cosd": cos, "sind": sin, "identd": ident,
        })
    return in_maps


def kernel(x, w_qkv, w_proj, b_proj, freqs_cos, freqs_sin):
    in_maps = _prep_inputs(x, w_qkv, w_proj, b_proj, freqs_cos, freqs_sin)
    nc = _get_nc()
    res = run_bass_kernel_spmd(nc, in_maps, core_ids=list(range(N_CORES)))
    full = np.empty((NTOT, C), dtype=np.float32)
    for i in range(N_CORES):
        o = res.results[i]["out"]
        full[HALF * i:HALF * (i + 1)] = o[0:HALF]
        full[N + HALF * i:N + HALF * (i + 1)] = o[HALF:RPC]
    return full.reshape(B, N, C).astype(np.float32)
